# revision 1
# baseline (speedup 1.0000x reference)
"""Causal single-head attention (B=4, T=4096, C=1024, H=64) on 8 TRN2 NeuronCores.

Sharding: core = 2*b + h handles batch b, t-half h (rows [h*2048, (h+1)*2048)).
Uniform SPMD program per core:
  - triangle: causal attention within the own t-half (s, t both in own half)
  - rect: S^T[s in [0,2048), t in [2048+off, 2048+off+1024)], off = (pid%2)*1024
    (the lower-half keys attending into the upper-half queries, split by t)
Pair AllGathers exchange qT, kT, v; rect partials (num|den) go through a 4th
AllGather and are added (gated by a per-core flag) before the final divide.
Softmax uses no max-subtraction (logits are O(6)); denominator comes from an
appended ones-column in v during the AV matmul.
"""
import sys

sys.path.insert(0, "/opt/trn_rl_repo")

from contextlib import ExitStack

import numpy as np

import concourse.bass as bass
import concourse.mybir as mybir
import concourse.tile as tile
from concourse import bacc
from concourse.bass_utils import run_bass_kernel_spmd

B, T, C, H = 4, 4096, 1024, 64
P = 128
HALF = T // 2              # 2048 rows per core
NB_C = C // P              # 8 contraction tiles
NT = HALF // P             # 16 own t/s tiles
RW = 1024                  # rect t-width per core
NRT = RW // P              # 8 rect t-tiles
SCALE = float(H) ** -0.5
NEG = -1e9
F32, F32R, BF16 = mybir.dt.float32, mybir.dt.float32r, mybir.dt.bfloat16
N_CORES = 8
PAIRS = [[2 * b, 2 * b + 1] for b in range(B)]

# triangle attT storage: s-tile i holds local t-cols [base_i, 2048)
TRI_BASE = [(i // 4) * 512 for i in range(NT)]
TRI_W = [HALF - b for b in TRI_BASE]
TRI_OFF = np.concatenate([[0], np.cumsum(TRI_W)]).tolist()
TRI_TOTAL = TRI_OFF[-1]  # 20480

_CACHE = {}
BODY_REPEAT = 1            # for differential timing in bench.py
PHASES = set(range(1, 9))  # ablation for phase timing
SCHEDULE = None            # list of phase-sets, one body emission each


def build():
    nc = bacc.Bacc("TRN2", target_bir_lowering=False, debug=False,
                   num_devices=N_CORES)
    x = nc.dram_tensor("x", [HALF, C], F32, kind="ExternalInput").ap()
    wq = nc.dram_tensor("wq", [C, H], F32, kind="ExternalInput").ap()
    wk = nc.dram_tensor("wk", [C, H], F32, kind="ExternalInput").ap()
    wv = nc.dram_tensor("wv", [C, H], F32, kind="ExternalInput").ap()
    flag = nc.dram_tensor("flag", [P, 1], F32, kind="ExternalInput").ap()
    trimask = nc.dram_tensor("trimask", [P, P], F32, kind="ExternalInput").ap()
    out = nc.dram_tensor("out", [HALF, H], F32, kind="ExternalOutput").ap()

    with tile.TileContext(nc) as tc, ExitStack() as ctx:
        sb = ctx.enter_context(tc.tile_pool(name="sb", bufs=2))
        stage = ctx.enter_context(tc.tile_pool(name="stage", bufs=4))
        big = ctx.enter_context(tc.tile_pool(name="big", bufs=1))
        ps = ctx.enter_context(tc.tile_pool(name="ps", bufs=2, space="PSUM"))
        dram = ctx.enter_context(tc.tile_pool(name="dram", bufs=1, space="DRAM"))

        # ---- constants ----
        tri_sb = big.tile([P, P], F32, tag="tri")
        nc.sync.dma_start(tri_sb[:], trimask[:])
        flag_sb = big.tile([P, 1], F32, tag="flag")
        nc.sync.dma_start(flag_sb[:], flag[:])
        wqk_sb = big.tile([P, NB_C, 2 * H], BF16, tag="wqk")
        nc.gpsimd.dma_start(wqk_sb[:, :, 0:H], wq.rearrange("(cb p) h -> p cb h", p=P))
        nc.gpsimd.dma_start(wqk_sb[:, :, H:2 * H], wk.rearrange("(cb p) h -> p cb h", p=P))
        wv_sb = big.tile([P, NB_C, H], BF16, tag="wv")
        nc.gpsimd.dma_start(wv_sb[:], wv.rearrange("(cb p) h -> p cb h", p=P))

        schedule = SCHEDULE if SCHEDULE is not None else [PHASES] * BODY_REPEAT
        for _rep in range(len(schedule)):
            cur = schedule[_rep]
            if 1 in cur:
                # ---- x transpose path: per-column-slab cast to bf16 in DRAM
                # (contiguous slab layout), then contiguous DMA-transpose ----
                xbf = dram.tile([NB_C, HALF, P], BF16)
                xT = big.tile([P, NB_C, HALF], BF16, tag="xT")
                for cb in range(NB_C):
                    nc.gpsimd.dma_start(xbf[cb], x[:, cb * P:(cb + 1) * P])
                    nc.sync.dma_start(xT[:, cb, :], xbf[cb], transpose=True)

            if 2 in cur:
                # ---- qk projection: qkT[0:64]=qT, [64:128]=kT (fp32r) ----
                qkT = big.tile([P, HALF], F32R, tag="qkT")
                for tg in range(4):
                    pqk = ps.tile([P, 512], F32, tag="ps")
                    for cb in range(NB_C):
                        nc.tensor.matmul(pqk[:], wqk_sb[:, cb, :],
                                         xT[:, cb, tg * 512:(tg + 1) * 512],
                                         start=(cb == 0), stop=(cb == NB_C - 1))
                    nc.vector.tensor_copy(qkT[:, tg * 512:(tg + 1) * 512], pqk[:])

                # ---- v projection (v_sb[:, st, 0:64]=v, col 64 = ones) ----
                v_sb = big.tile([P, NT, H + 2], BF16, tag="v")
                nc.vector.memset(v_sb[:, :, H:H + 1], 1.0)
                for st in range(NT):
                    pv = ps.tile([P, H], F32, tag="ps")
                    for cb in range(NB_C):
                        nc.tensor.matmul(pv[:], xT[:, cb, st * P:(st + 1) * P],
                                         wv_sb[:, cb, :],
                                         start=(cb == 0), stop=(cb == NB_C - 1))
                    nc.vector.tensor_copy(v_sb[:, st, 0:H], pv[:])

                # kT relocated to partitions 0:64 (matmul needs same base)
                kT = big.tile([H, HALF], F32R, tag="kT")
                nc.sync.dma_start(kT[:], qkT[H:P, :])

            if 3 in cur:
                # ---- pair collectives: gather qT, kT, v ----
                qb = dram.tile([H, HALF], F32R)
                kb = dram.tile([H, HALF], F32R)
                vb = dram.tile([HALF, H], BF16)
                nc.sync.dma_start(qb[:], qkT[0:H, :])
                nc.sync.dma_start(kb[:], qkT[H:P, :])
                nc.sync.dma_start(vb.rearrange("(st p) h -> p st h", p=P),
                                  v_sb[:, :, 0:H])
                gq = dram.tile([2 * H, HALF], F32R)
                gk = dram.tile([2 * H, HALF], F32R)
                gv = dram.tile([T, H], BF16)
                for src, dst in ((qb, gq), (kb, gk), (vb, gv)):
                    nc.gpsimd.collective_compute(
                        "AllGather", mybir.AluOpType.bypass, replica_groups=PAIRS,
                        ins=[src.opt()], outs=[dst.opt()])

                # rect operands: KR/VR = lower-half kT/v (rank0), QR = rank1 qT
                # cols [off, off+RW), off = (pid%2)*RW (dynamic)
                KR = big.tile([H, HALF], F32R, tag="KR")
                nc.sync.dma_start(KR[:], gk[0:H, :])
                VR = big.tile([P, NT, H + 2], BF16, tag="VR")
                nc.vector.memset(VR[:, :, H:H + 1], 1.0)
                nc.sync.dma_start(VR[:, :, 0:H],
                                  gv[0:HALF, :].rearrange("(st p) h -> p st h", p=P))
                QR = big.tile([H, RW], F32R, tag="QR")
                pid = nc.partition_id(engines=[mybir.EngineType.Pool])
                qoff = (pid % 2) * RW
                nc.gpsimd.dma_start(QR[:], gq[H:2 * H, bass.ds(qoff, RW)])

            if 5 in cur:
                # ---- rect QK^T + exp (no mask: s < t always) ----
                attT_rect = big.tile([P, NT * RW], BF16, tag="att_rect")
                for i in range(NT):
                    psr = ps.tile([P, RW], F32, tag="ps")
                    for g in range(RW // 512):
                        nc.tensor.matmul(psr[:, g * 512:(g + 1) * 512],
                                         KR[:, i * P:(i + 1) * P],
                                         QR[:, g * 512:(g + 1) * 512],
                                         start=True, stop=True)
                    nc.scalar.activation(attT_rect[:, i * RW:(i + 1) * RW], psr[:],
                                         mybir.ActivationFunctionType.Exp, scale=SCALE)

            if 7 in cur:
                # ---- rect AV -> partials ----
                rectnd = big.tile([P, NRT, H + 2], F32, tag="rectnd")
                for rt in range(NRT):
                    pr = ps.tile([P, H + 1], F32, tag="ps")
                    for st in range(NT):
                        col = st * RW + rt * P
                        nc.tensor.matmul(pr[:], attT_rect[:, col:col + P],
                                         VR[:, st, 0:H + 1],
                                         start=(st == 0), stop=(st == NT - 1))
                    nc.vector.tensor_copy(rectnd[:, rt, 0:H + 1], pr[:])

            if 8 in cur:
                # ---- partial exchange (flies during the triangle phases) ----
                ndb = dram.tile([RW, H + 2], F32)
                nc.sync.dma_start(ndb.rearrange("(rt p) h -> p rt h", p=P), rectnd[:])
                gnd = dram.tile([HALF, H + 2], F32)
                nc.gpsimd.collective_compute(
                    "AllGather", mybir.AluOpType.bypass, replica_groups=PAIRS,
                    ins=[ndb.opt()], outs=[gnd.opt()])
                gnd_sb = big.tile([P, NT, H + 2], F32, tag="gnd")
                nc.sync.dma_start(gnd_sb[:], gnd.rearrange("(tt p) h -> p tt h", p=P))

            if 4 in cur:
                # ---- triangle QK^T (S^T layout) + exp ----
                attT_tri = big.tile([P, TRI_TOTAL], BF16, tag="att_tri")
                for i in range(NT):
                    base, w = TRI_BASE[i], TRI_W[i]
                    pst = ps.tile([P, w], F32, tag="ps")
                    for g in range(base // 512, 4):
                        nc.tensor.matmul(pst[:, g * 512 - base:(g + 1) * 512 - base],
                                         kT[:, i * P:(i + 1) * P],
                                         qkT[0:H, g * 512:(g + 1) * 512],
                                         start=True, stop=True)
                    d0 = i * P - base
                    nc.vector.tensor_add(pst[:, d0:d0 + P], pst[:, d0:d0 + P], tri_sb[:])
                    nc.scalar.activation(attT_tri[:, TRI_OFF[i]:TRI_OFF[i] + w], pst[:],
                                         mybir.ActivationFunctionType.Exp, scale=SCALE)

            if 6 in cur:
                # ---- triangle AV (num|den via ones column) ----
                trind = big.tile([P, NT, H + 2], F32, tag="trind")
                for tt in range(NT):
                    po = ps.tile([P, H + 1], F32, tag="ps")
                    for st in range(tt + 1):
                        col = TRI_OFF[st] + tt * P - TRI_BASE[st]
                        nc.tensor.matmul(po[:], attT_tri[:, col:col + P],
                                         v_sb[:, st, 0:H + 1],
                                         start=(st == 0), stop=(st == tt))
                    nc.vector.tensor_copy(trind[:, tt, 0:H + 1], po[:])

            if 8 in cur:
                # ---- final: nd = tri + flag*gathered; out = num/den ----
                for tt in range(NT):
                    tmp = sb.tile([P, H + 1], F32, tag="tmp")
                    nc.vector.tensor_scalar_mul(tmp[:], gnd_sb[:, tt, 0:H + 1],
                                                flag_sb[:, 0:1])
                    ndf = sb.tile([P, H + 1], F32, tag="ndf")
                    nc.vector.tensor_add(ndf[:], trind[:, tt, 0:H + 1], tmp[:])
                    rec = sb.tile([P, 1], F32, tag="rec")
                    nc.vector.reciprocal(rec[:], ndf[:, H:H + 1])
                    ot = sb.tile([P, H], F32, tag="ot")
                    nc.vector.tensor_scalar_mul(ot[:], ndf[:, 0:H], rec[:, 0:1])
                    nc.sync.dma_start(out[tt * P:(tt + 1) * P, :], ot[:])

    nc.compile()
    return nc


def kernel(x, Wq, Wk, Wv):
    x = np.asarray(x, dtype=np.float32)
    Wq = np.asarray(Wq, dtype=np.float32)
    Wk = np.asarray(Wk, dtype=np.float32)
    Wv = np.asarray(Wv, dtype=np.float32)
    if "nc" not in _CACHE:
        _CACHE["nc"] = build()
    nc = _CACHE["nc"]

    # S^T layout: partition=s, free=t; allowed s<=t -> tri[s,t]=0 iff s<=t
    tri = np.where(np.arange(P)[:, None] <= np.arange(P)[None, :], 0.0,
                   NEG).astype(np.float32)
    in_maps = []
    for c in range(N_CORES):
        b, h = c // 2, c % 2
        in_maps.append({
            "x": np.ascontiguousarray(x[b, h * HALF:(h + 1) * HALF, :]),
            "wq": Wq, "wk": Wk, "wv": Wv,
            "flag": np.full((P, 1), float(h), np.float32),
            "trimask": tri,
        })
    res = None
    for attempt in range(4):
        try:
            res = run_bass_kernel_spmd(nc, in_maps, list(range(N_CORES)))
            break
        except Exception:
            if attempt == 3:
                raise
            import time as _time
            _time.sleep(5)
    out = np.empty((B, T, H), np.float32)
    for c in range(N_CORES):
        b, h = c // 2, c % 2
        out[b, h * HALF:(h + 1) * HALF, :] = res.results[c]["out"]
    return out



# revision 20
# speedup vs baseline: 1.0801x; 1.0801x over previous
"""Causal single-head attention (B=4, T=4096, C=1024, H=64) on 8 TRN2 NeuronCores.

Sharding: core = 2*b + h handles batch b, t-half h (rows [h*2048, (h+1)*2048)).
Uniform SPMD program per core:
  - triangle: causal attention within the own t-half (s, t both in own half)
  - rect: S^T[s in [0,2048), t in [2048+off, 2048+off+1024)], off = (pid%2)*1024
    (lower-half keys attending into upper-half queries, t-split across the pair)

v2 design:
  - x loaded per t-tile with SWDGE cast f32->bf16 into SBUF, then one 3-D xbar
    SBUF->SBUF DMA transpose per tile: xT[p, cb, t] = x[t, 128*cb + p].
  - v projected in vT form (wide moving operand), then xbar-transposed into
    v_own[s, h] layout with an appended ones column (softmax denominator).
  - QK^T runs as row-packed pairs: two concurrent K=64 matmuls on row groups
    (0,0)/(64,0), needing q/k duplicated into both partition halves.
  - AV computed transposed: outT[h, t] += v'[s, h].T @ attT[s, t] with 512-wide
    bf16 moving operand; row 64 of the accumulator is the denominator.
  - Rect partials pair-AllGathered; merged into trind by a conditional
    accumulate-DMA (only on the upper-half core). Final divide via reciprocal
    of the den row + K=1 outer-product broadcast matmul. Output written as
    outT [64, 2048]; the host transposes.
Softmax uses no max-subtraction (logits are O(6)).
"""
import sys

sys.path.insert(0, "/opt/trn_rl_repo")

from contextlib import ExitStack

import numpy as np

import concourse.bass as bass
import concourse.mybir as mybir
import concourse.tile as tile
from concourse import bacc
from concourse.bass_utils import run_bass_kernel_spmd

B, T, C, H = 4, 4096, 1024, 64
P = 128
HALF = T // 2              # 2048 rows per core
NB_C = C // P              # 8 contraction tiles
NT = HALF // P             # 16 own t/s tiles
RW = 1024                  # rect t-width per core
SCALE = float(H) ** -0.5
NEG = -1e9
F32, F32R, BF16 = mybir.dt.float32, mybir.dt.float32r, mybir.dt.bfloat16
N_CORES = 8
PAIRS = [[2 * b, 2 * b + 1] for b in range(B)]

# triangle attT storage: s-tile i holds t-cols [256*(i//2), 2048)
TRI_BASE = [256 * (i // 2) for i in range(NT)]
TRI_W = [HALF - b for b in TRI_BASE]
TRI_OFF = np.concatenate([[0], np.cumsum(TRI_W)]).tolist()
TRI_TOTAL = TRI_OFF[-1]  # 18432
VROW = H + 16               # v_own/v_rect row stride: 160B, 32B-aligned

_CACHE = {}
BODY_REPEAT = 1            # for differential timing in test.py
PHASES = set(range(1, 10))  # ablation for phase timing
SCHEDULE = None            # list of phase-sets, one body emission each
DEBUG_DUMPS = False        # emit intermediate tensors as extra outputs


def build():
    nc = bacc.Bacc("TRN2", target_bir_lowering=False, debug=False,
                   num_devices=N_CORES)
    x = nc.dram_tensor("x", [HALF, C], F32, kind="ExternalInput").ap()
    wq = nc.dram_tensor("wq", [C, H], F32, kind="ExternalInput").ap()
    wk = nc.dram_tensor("wk", [C, H], F32, kind="ExternalInput").ap()
    wv = nc.dram_tensor("wv", [C, H], F32, kind="ExternalInput").ap()
    trimask = nc.dram_tensor("trimask", [P, P], F32, kind="ExternalInput").ap()
    outT = nc.dram_tensor("outT", [H, HALF], F32, kind="ExternalOutput").ap()

    EXP = mybir.ActivationFunctionType.Exp

    with tile.TileContext(nc) as tc, ExitStack() as ctx:
        big = ctx.enter_context(tc.tile_pool(name="big", bufs=1))
        stage = ctx.enter_context(tc.tile_pool(name="stage", bufs=2))
        ps = ctx.enter_context(tc.tile_pool(name="ps", bufs=2, space="PSUM"))
        dram = ctx.enter_context(tc.tile_pool(name="dram", bufs=1, space="DRAM"))

        # ---- constants ----
        tri_sb = big.tile([P, P], F32, tag="tri")
        nc.sync.dma_start(tri_sb[:], trimask[:])
        wqk_sb = big.tile([P, NB_C, 2 * H], BF16, tag="wqk")
        nc.gpsimd.dma_start(wqk_sb[:, :, 0:H], wq.rearrange("(cb p) h -> p cb h", p=P))
        nc.gpsimd.dma_start(wqk_sb[:, :, H:2 * H], wk.rearrange("(cb p) h -> p cb h", p=P))
        wv_sb = big.tile([P, NB_C, H], BF16, tag="wv")
        nc.gpsimd.dma_start(wv_sb[:], wv.rearrange("(cb p) h -> p cb h", p=P))
        # ones row on partition H (=64): lhsT for the den-broadcast matmul must
        # share the contraction partition with the den row of trind
        ones_sb = big.tile([H + 1, H], F32, tag="ones")
        nc.vector.memset(ones_sb[H:H + 1, :], 1.0)
        ones_r = big.tile([H + 1, H], F32R, tag="ones_r")
        nc.vector.tensor_copy(ones_r[H:H + 1, :], ones_sb[H:H + 1, :])

        pid = nc.partition_id(engines=[mybir.EngineType.Pool])
        qoff = (pid % 2) * RW
        is_h1 = pid % 2

        schedule = SCHEDULE if SCHEDULE is not None else [PHASES] * BODY_REPEAT
        for _rep in range(len(schedule)):
            cur = schedule[_rep]
            if 1 in cur:
                # ---- x load (cast bf16) + 3-D xbar transpose per t-tile ----
                xT = big.tile([P, NB_C, HALF], BF16, tag="xT")
                for tt in range(NT):
                    xs = stage.tile([P, C], BF16, tag="xs")
                    nc.gpsimd.dma_start(xs[:], x[tt * P:(tt + 1) * P, :])
                    nc.sync.dma_start(xT[:, :, tt * P:(tt + 1) * P], xs[:],
                                      transpose=True)

            if 2 in cur:
                # ---- projections: q|k packed; v in vT form ----
                qdup = big.tile([P, HALF], F32R, tag="qdup")
                kdup = big.tile([P, HALF], F32R, tag="kdup")
                vT_sb = big.tile([H, HALF], BF16, tag="vT")
                for tg in range(4):
                    sl = slice(tg * 512, (tg + 1) * 512)
                    pqk = ps.tile([P, 512], F32, tag="a")
                    for cb in range(NB_C):
                        nc.tensor.matmul(pqk[:], wqk_sb[:, cb, :],
                                         xT[:, cb, sl],
                                         start=(cb == 0), stop=(cb == NB_C - 1))
                    nc.vector.tensor_copy(qdup[0:H, sl], pqk[0:H, :])
                    nc.vector.tensor_copy(kdup[H:P, sl], pqk[H:P, :])
                    pv = ps.tile([H, 512], F32, tag="v")
                    for cb in range(NB_C):
                        nc.tensor.matmul(pv[:], wv_sb[:, cb, :],
                                         xT[:, cb, sl],
                                         start=(cb == 0), stop=(cb == NB_C - 1))
                    nc.vector.tensor_copy(vT_sb[:, sl], pv[:])
                # q/k/vT out to DRAM (feeds both the AllGather and the dups);
                # vT goes out as 16 contiguous [H, P] blocks — the xbar
                # transpose needs contiguous DRAM sources
                qb = dram.tile([H, HALF], F32R)
                kb = dram.tile([H, HALF], F32R)
                vb3 = dram.tile([NT, H, P], BF16)
                nc.gpsimd.dma_start(qb[:], qdup[0:H, :])
                nc.gpsimd.dma_start(kb[:], kdup[H:P, :])
                for tt in range(NT):
                    nc.gpsimd.dma_start(vb3[tt], vT_sb[:, tt * P:(tt + 1) * P])
                # v into [s, h] layout with ones column (xbar per block)
                v_own = big.tile([P, NT, VROW], BF16, tag="vown")
                nc.vector.memset(v_own[:, :, H:H + 1], 1.0)
                for tt in range(NT):
                    nc.scalar.dma_start(v_own[:, tt, 0:H], vb3[tt],
                                        transpose=True)
                # cross-partition dups via DRAM (DMA->DMA deps track reliably)
                nc.sync.dma_start(qdup[H:P, :], qb[:])
                nc.sync.dma_start(kdup[0:H, :], kb[:])

            if 3 in cur:
                # ---- pair collectives: gather qT, kT, vT ----
                # the AG gets its own copy of vb3 — collective reads of the
                # same buffer corrupt concurrent xbar-transpose reads
                vb3g = dram.tile([NT, H, P], BF16)
                nc.gpsimd.dma_start(vb3g[:], vb3[:])
                gq = dram.tile([2 * H, HALF], F32R)
                gk = dram.tile([2 * H, HALF], F32R)
                gv3 = dram.tile([2 * NT, H, P], BF16)
                for src, dst in ((qb, gq), (kb, gk), (vb3g, gv3)):
                    nc.gpsimd.collective_compute(
                        "AllGather", mybir.AluOpType.bypass,
                        replica_groups=PAIRS,
                        ins=[src.opt()], outs=[dst.opt()])

            if 4 in cur:
                # ---- triangle QK^T as row-packed pairs + exp ----
                attT_tri = big.tile([P, TRI_TOTAL], BF16, tag="att_tri")
                for j in range(8):
                    base = 256 * j
                    i0, i1 = 2 * j, 2 * j + 1
                    for c0 in range(base, HALF, 512):
                        w = min(512, HALF - c0)
                        pa = ps.tile([P, 512], F32, tag="a")
                        pb = ps.tile([P, 512], F32, tag="b")
                        nc.tensor.matmul(pa[:, 0:w],
                                         kdup[0:H, i0 * P:(i0 + 1) * P],
                                         qdup[0:H, c0:c0 + w],
                                         start=True, stop=True)
                        nc.tensor.matmul(pb[:, 0:w],
                                         kdup[H:P, i1 * P:(i1 + 1) * P],
                                         qdup[H:P, c0:c0 + w],
                                         start=True, stop=True)
                        if c0 == base:
                            nc.vector.tensor_add(pa[:, 0:P], pa[:, 0:P], tri_sb[:])
                            nc.vector.tensor_add(pb[:, P:2 * P], pb[:, P:2 * P],
                                                 tri_sb[:])
                        d0 = c0 - base
                        nc.scalar.activation(
                            attT_tri[:, TRI_OFF[i0] + d0:TRI_OFF[i0] + d0 + w],
                            pa[:, 0:w], EXP, scale=SCALE)
                        nc.scalar.activation(
                            attT_tri[:, TRI_OFF[i1] + d0:TRI_OFF[i1] + d0 + w],
                            pb[:, 0:w], EXP, scale=SCALE)

            if 5 in cur:
                # ---- rect operands (from gathered) + rect QK^T pairs ----
                kdup_r = big.tile([P, HALF], F32R, tag="kdup_r")
                nc.sync.dma_start(kdup_r[0:H, :], gk[0:H, :])
                nc.sync.dma_start(kdup_r[H:P, :], gk[0:H, :])
                qdup_r = big.tile([P, RW], F32R, tag="qdup_r")
                nc.gpsimd.dma_start(qdup_r[0:H, :], gq[H:2 * H, bass.ds(qoff, RW)])
                nc.gpsimd.dma_start(qdup_r[H:P, :], gq[H:2 * H, bass.ds(qoff, RW)])
                v_rect = big.tile([P, NT, VROW], BF16, tag="vrect")
                nc.vector.memset(v_rect[:, :, H:H + 1], 1.0)
                for tt in range(NT):
                    nc.scalar.dma_start(v_rect[:, tt, 0:H], gv3[tt],
                                        transpose=True)

                attT_rect = big.tile([P, NT * RW], BF16, tag="att_rect")
                for jp in range(8):
                    i0, i1 = 2 * jp, 2 * jp + 1
                    for g in range(2):
                        sl = slice(g * 512, (g + 1) * 512)
                        pa = ps.tile([P, 512], F32, tag="a")
                        pb = ps.tile([P, 512], F32, tag="b")
                        nc.tensor.matmul(pa[:], kdup_r[0:H, i0 * P:(i0 + 1) * P],
                                         qdup_r[0:H, sl], start=True, stop=True)
                        nc.tensor.matmul(pb[:], kdup_r[H:P, i1 * P:(i1 + 1) * P],
                                         qdup_r[H:P, sl], start=True, stop=True)
                        nc.scalar.activation(
                            attT_rect[:, i0 * RW + g * 512:i0 * RW + g * 512 + 512],
                            pa[:], EXP, scale=SCALE)
                        nc.scalar.activation(
                            attT_rect[:, i1 * RW + g * 512:i1 * RW + g * 512 + 512],
                            pb[:], EXP, scale=SCALE)

            if 7 in cur:
                # ---- rect AV (transposed, num|den via ones row) ----
                rectnd = big.tile([H + 1, RW], F32, tag="rectnd")
                for g in range(2):
                    pav = ps.tile([H + 1, 512], F32, tag="a")
                    for st in range(NT):
                        nc.tensor.matmul(
                            pav[:], v_rect[:, st, 0:H + 1],
                            attT_rect[:, st * RW + g * 512:st * RW + g * 512 + 512],
                            start=(st == 0), stop=(st == NT - 1))
                    nc.vector.tensor_copy(rectnd[:, g * 512:(g + 1) * 512], pav[:])

            if 8 in cur:
                # ---- partial exchange (flies during the triangle AV) ----
                ndb = dram.tile([H + 1, RW], F32)
                nc.gpsimd.dma_start(ndb[:], rectnd[:])
                gnd = dram.tile([2 * (H + 1), RW], F32)
                nc.gpsimd.collective_compute(
                    "AllGather", mybir.AluOpType.bypass, replica_groups=PAIRS,
                    ins=[ndb.opt()], outs=[gnd.opt()])

            if 6 in cur:
                # ---- triangle AV (transposed accumulation) ----
                trind = big.tile([H + 1, HALF], F32, tag="trind")
                for g in range(4):
                    pav = ps.tile([H + 1, 512], F32, tag="b")
                    last = 4 * g + 3
                    for st in range(last + 1):
                        cs = max(512 * g, P * st)
                        w = 512 * g + 512 - cs
                        col = TRI_OFF[st] + cs - TRI_BASE[st]
                        nc.tensor.matmul(pav[:, cs - 512 * g:512],
                                         v_own[:, st, 0:H + 1],
                                         attT_tri[:, col:col + w],
                                         start=(st == 0), stop=(st == last))
                    nc.vector.tensor_copy(trind[:, g * 512:(g + 1) * 512], pav[:])

            if 9 in cur:
                # ---- merge rect partials (upper core only) + divide ----
                nc.gpsimd.dma_start(trind[:, 0:RW], gnd[0:H + 1, :],
                                    accum_op=mybir.AluOpType.add, cond=is_h1)
                nc.gpsimd.dma_start(trind[:, RW:HALF], gnd[H + 1:2 * (H + 1), :],
                                    accum_op=mybir.AluOpType.add, cond=is_h1)
                nc.vector.reciprocal(trind[H:H + 1, :], trind[H:H + 1, :])
                recip_r = big.tile([H + 1, HALF], F32R, tag="recip_r")
                nc.vector.tensor_copy(recip_r[H:H + 1, :], trind[H:H + 1, :])
                out_sb = big.tile([H, HALF], F32, tag="out_sb")
                for g in range(4):
                    sl = slice(g * 512, (g + 1) * 512)
                    pbc = ps.tile([H, 512], F32, tag="v")
                    nc.tensor.matmul(pbc[:], ones_r[H:H + 1, :],
                                     recip_r[H:H + 1, sl],
                                     start=True, stop=True)
                    nc.vector.tensor_mul(out_sb[:, sl], trind[0:H, sl], pbc[:])
                nc.sync.dma_start(outT[:], out_sb[:])

            if DEBUG_DUMPS:
                def dump(name, src_fn, shape, dt):
                    try:
                        src_ap = src_fn()
                    except NameError:
                        return
                    o = nc.dram_tensor(name, shape, dt,
                                       kind="ExternalOutput").ap()
                    nc.sync.dma_start(o[:], src_ap)
                dump("d_qdup", lambda: qdup[:], [P, HALF], F32R)
                dump("d_kdup", lambda: kdup[:], [P, HALF], F32R)
                dump("d_vT", lambda: vT_sb[:], [H, HALF], BF16)
                dump("d_vown", lambda: v_own[:].rearrange("p a b -> p (a b)"),
                     [P, NT * VROW], BF16)
                dump("d_atttri0", lambda: attT_tri[:, 0:2048], [P, 2048], BF16)
                dump("d_attrect0", lambda: attT_rect[:, 0:RW], [P, RW], BF16)
                dump("d_rectnd", lambda: rectnd[:], [H + 1, RW], F32)
                dump("d_trind", lambda: trind[:], [H + 1, HALF], F32)
                dump("d_vb3", lambda: vb3[:], [NT, H, P], BF16)

    nc.compile()
    return nc


def make_in_maps(x, Wq, Wk, Wv):
    x = np.asarray(x, dtype=np.float32)
    Wq = np.asarray(Wq, dtype=np.float32)
    Wk = np.asarray(Wk, dtype=np.float32)
    Wv = np.asarray(Wv, dtype=np.float32)
    # S^T layout: partition=s, free=t; allowed s<=t -> tri[s,t]=0 iff s<=t
    tri = np.where(np.arange(P)[:, None] <= np.arange(P)[None, :], 0.0,
                   NEG).astype(np.float32)
    in_maps = []
    for c in range(N_CORES):
        b, h = c // 2, c % 2
        in_maps.append({
            "x": np.ascontiguousarray(x[b, h * HALF:(h + 1) * HALF, :]),
            "wq": Wq, "wk": Wk, "wv": Wv,
            "trimask": tri,
        })
    return in_maps


def kernel(x, Wq, Wk, Wv):
    if "nc" not in _CACHE:
        _CACHE["nc"] = build()
    nc = _CACHE["nc"]
    in_maps = make_in_maps(x, Wq, Wk, Wv)
    res = None
    for attempt in range(4):
        try:
            res = run_bass_kernel_spmd(nc, in_maps, list(range(N_CORES)))
            break
        except Exception:
            if attempt == 3:
                raise
            import time as _time
            _time.sleep(5)
    out = np.empty((B, T, H), np.float32)
    for c in range(N_CORES):
        b, h = c // 2, c % 2
        out[b, h * HALF:(h + 1) * HALF, :] = res.results[c]["outT"].T
    return out


# revision 23
# speedup vs baseline: 2.1082x; 1.9519x over previous
"""Causal single-head attention (B=4, T=4096, C=1024, H=64) on 8 TRN2 NeuronCores.

Sharding: core = 2*b + h handles batch b, t-half h (rows [h*2048, (h+1)*2048)).
Uniform SPMD program per core:
  - triangle: causal attention within the own t-half (s, t both in own half)
  - rect: S^T[s in [0,2048), t in [2048+off, 2048+off+1024)], off = (pid%2)*1024
    (lower-half keys attending into upper-half queries, t-split across the pair)

v2 design:
  - x loaded per t-tile with SWDGE cast f32->bf16 into SBUF, then one 3-D xbar
    SBUF->SBUF DMA transpose per tile: xT[p, cb, t] = x[t, 128*cb + p].
  - v projected in vT form (wide moving operand), then xbar-transposed into
    v_own[s, h] layout with an appended ones column (softmax denominator).
  - QK^T runs as row-packed pairs: two concurrent K=64 matmuls on row groups
    (0,0)/(64,0), needing q/k duplicated into both partition halves.
  - AV computed transposed: outT[h, t] += v'[s, h].T @ attT[s, t] with 512-wide
    bf16 moving operand; row 64 of the accumulator is the denominator.
  - Rect partials pair-AllGathered; merged into trind by a conditional
    accumulate-DMA (only on the upper-half core). Final divide via reciprocal
    of the den row + K=1 outer-product broadcast matmul. Output written as
    outT [64, 2048]; the host transposes.
Softmax uses no max-subtraction (logits are O(6)).
"""
import sys

sys.path.insert(0, "/opt/trn_rl_repo")

from contextlib import ExitStack

import numpy as np

import concourse.bass as bass
import concourse.mybir as mybir
import concourse.tile as tile
from concourse import bacc
from concourse.bass_utils import run_bass_kernel_spmd

B, T, C, H = 4, 4096, 1024, 64
P = 128
HALF = T // 2              # 2048 rows per core
NB_C = C // P              # 8 contraction tiles
NT = HALF // P             # 16 own t/s tiles
RW = 1024                  # rect t-width per core
SCALE = float(H) ** -0.5
NEG = -1e9
F32, F32R, BF16 = mybir.dt.float32, mybir.dt.float32r, mybir.dt.bfloat16
N_CORES = 8
PAIRS = [[2 * b, 2 * b + 1] for b in range(B)]

# triangle attT storage: s-tile i holds t-cols [256*(i//2), 2048)
TRI_BASE = [256 * (i // 2) for i in range(NT)]
TRI_W = [HALF - b for b in TRI_BASE]
TRI_OFF = np.concatenate([[0], np.cumsum(TRI_W)]).tolist()
TRI_TOTAL = TRI_OFF[-1]  # 18432
VROW = H + 16               # v_own/v_rect row stride: 160B, 32B-aligned

_CACHE = {}
BODY_REPEAT = 1            # for differential timing in test.py
PHASES = set(range(1, 10))  # ablation for phase timing
SCHEDULE = None            # list of phase-sets, one body emission each
DEBUG_DUMPS = False        # emit intermediate tensors as extra outputs


def build():
    nc = bacc.Bacc("TRN2", target_bir_lowering=False, debug=False,
                   num_devices=N_CORES)
    x = nc.dram_tensor("x", [HALF, C], F32, kind="ExternalInput").ap()
    wq = nc.dram_tensor("wq", [C, H], F32, kind="ExternalInput").ap()
    wk = nc.dram_tensor("wk", [C, H], F32, kind="ExternalInput").ap()
    wv = nc.dram_tensor("wv", [C, H], F32, kind="ExternalInput").ap()
    trimask = nc.dram_tensor("trimask", [P, P], F32, kind="ExternalInput").ap()
    outT = nc.dram_tensor("outT", [H, HALF], F32, kind="ExternalOutput").ap()

    EXP = mybir.ActivationFunctionType.Exp

    with tile.TileContext(nc) as tc, ExitStack() as ctx:
        big = ctx.enter_context(tc.tile_pool(name="big", bufs=1))
        stage = ctx.enter_context(tc.tile_pool(name="stage", bufs=2))
        ps = ctx.enter_context(tc.tile_pool(name="ps", bufs=2, space="PSUM"))
        dram = ctx.enter_context(tc.tile_pool(name="dram", bufs=1, space="DRAM"))

        # ---- constants ----
        tri_sb = big.tile([P, P], F32, tag="tri")
        nc.sync.dma_start(tri_sb[:], trimask[:])
        wqk_sb = big.tile([P, NB_C, 2 * H], BF16, tag="wqk")
        nc.gpsimd.dma_start(wqk_sb[:, :, 0:H], wq.rearrange("(cb p) h -> p cb h", p=P))
        nc.gpsimd.dma_start(wqk_sb[:, :, H:2 * H], wk.rearrange("(cb p) h -> p cb h", p=P))
        wv_sb = big.tile([P, NB_C, H], BF16, tag="wv")
        nc.gpsimd.dma_start(wv_sb[:], wv.rearrange("(cb p) h -> p cb h", p=P))
        # ones row on partition H (=64): lhsT for the den-broadcast matmul must
        # share the contraction partition with the den row of trind
        ones_sb = big.tile([H + 1, H], F32, tag="ones")
        nc.vector.memset(ones_sb[H:H + 1, :], 1.0)
        ones_r = big.tile([H + 1, H], F32R, tag="ones_r")
        nc.vector.tensor_copy(ones_r[H:H + 1, :], ones_sb[H:H + 1, :])

        pid = nc.partition_id(engines=[mybir.EngineType.Pool])
        qoff = (pid % 2) * RW
        is_h1 = pid % 2

        schedule = SCHEDULE if SCHEDULE is not None else [PHASES] * BODY_REPEAT
        for _rep in range(len(schedule)):
            cur = schedule[_rep]
            if 1 in cur:
                # ---- x load (cast bf16, 512-row quarters) + xbar per t-tile
                # (transposes alternate across both HWDGE rings) ----
                xT = big.tile([P, NB_C, HALF], BF16, tag="xT")
                for tt in range(NT):
                    xs = stage.tile([P, C], BF16, tag="xs")
                    nc.gpsimd.dma_start(xs[:], x[tt * P:(tt + 1) * P, :])
                    nc.sync.dma_start(xT[:, :, tt * P:(tt + 1) * P], xs[:],
                                      transpose=True)

            if 2 in cur:
                # ---- projections: q|k packed wide; v direct in [t, h] ----
                qdup = big.tile([P, HALF], F32R, tag="qdup")
                kdup = big.tile([P, HALF], F32R, tag="kdup")
                for tg in range(4):
                    sl = slice(tg * 512, (tg + 1) * 512)
                    pqk = ps.tile([P, 512], F32, tag="a")
                    for cb in range(NB_C):
                        nc.tensor.matmul(pqk[:], wqk_sb[:, cb, :],
                                         xT[:, cb, sl],
                                         start=(cb == 0), stop=(cb == NB_C - 1))
                    nc.vector.tensor_copy(qdup[0:H, sl], pqk[0:H, :])
                    nc.vector.tensor_copy(kdup[H:P, sl], pqk[H:P, :])
                v_own = big.tile([P, NT, VROW], BF16, tag="vown")
                nc.vector.memset(v_own[:, :, H:H + 1], 1.0)
                for st in range(NT):
                    pv = ps.tile([P, H], F32, tag="v")
                    for cb in range(NB_C):
                        nc.tensor.matmul(pv[:],
                                         xT[:, cb, st * P:(st + 1) * P],
                                         wv_sb[:, cb, :],
                                         start=(cb == 0), stop=(cb == NB_C - 1))
                    nc.vector.tensor_copy(v_own[:, st, 0:H], pv[:])
                # q|k and v out to DRAM (feeds the AllGather and the dups)
                qkb = dram.tile([2 * H, HALF], F32R)
                vb2 = dram.tile([HALF, H], BF16)
                nc.gpsimd.dma_start(qkb[0:H, :], qdup[0:H, :])
                nc.gpsimd.dma_start(qkb[H:2 * H, :], kdup[H:P, :])
                nc.gpsimd.dma_start(vb2.rearrange("(st p) h -> p st h", p=P),
                                    v_own[:, :, 0:H])
                # cross-partition dups via DRAM
                nc.sync.dma_start(qdup[H:P, :], qkb[0:H, :])
                nc.sync.dma_start(kdup[0:H, :], qkb[H:2 * H, :])

            if 3 in cur:
                # ---- pair collectives: gather q|k and v ----
                gqk = dram.tile([4 * H, HALF], F32R)
                gv2 = dram.tile([T, H], BF16)
                for src, dst in ((qkb, gqk), (vb2, gv2)):
                    nc.gpsimd.collective_compute(
                        "AllGather", mybir.AluOpType.bypass,
                        replica_groups=PAIRS,
                        ins=[src.opt()], outs=[dst.opt()])

            if 4 in cur:
                # ---- triangle QK^T as row-packed pairs + exp ----
                attT_tri = big.tile([P, TRI_TOTAL], BF16, tag="att_tri")
                for j in range(8):
                    base = 256 * j
                    i0, i1 = 2 * j, 2 * j + 1
                    for c0 in range(base, HALF, 512):
                        w = min(512, HALF - c0)
                        pa = ps.tile([P, 512], F32, tag="a")
                        pb = ps.tile([P, 512], F32, tag="b")
                        nc.tensor.matmul(pa[:, 0:w],
                                         kdup[0:H, i0 * P:(i0 + 1) * P],
                                         qdup[0:H, c0:c0 + w],
                                         start=True, stop=True)
                        nc.tensor.matmul(pb[:, 0:w],
                                         kdup[H:P, i1 * P:(i1 + 1) * P],
                                         qdup[H:P, c0:c0 + w],
                                         start=True, stop=True)
                        if c0 == base:
                            nc.vector.tensor_add(pa[:, 0:P], pa[:, 0:P], tri_sb[:])
                            nc.vector.tensor_add(pb[:, P:2 * P], pb[:, P:2 * P],
                                                 tri_sb[:])
                        d0 = c0 - base
                        nc.scalar.activation(
                            attT_tri[:, TRI_OFF[i0] + d0:TRI_OFF[i0] + d0 + w],
                            pa[:, 0:w], EXP, scale=SCALE)
                        nc.scalar.activation(
                            attT_tri[:, TRI_OFF[i1] + d0:TRI_OFF[i1] + d0 + w],
                            pb[:, 0:w], EXP, scale=SCALE)

            if 5 in cur:
                # ---- rect operands (from gathered) + rect QK^T pairs ----
                kdup_r = big.tile([P, HALF], F32R, tag="kdup_r")
                nc.sync.dma_start(kdup_r[0:H, :], gqk[H:2 * H, :])
                nc.sync.dma_start(kdup_r[H:P, :], gqk[H:2 * H, :])
                qdup_r = big.tile([P, RW], F32R, tag="qdup_r")
                nc.gpsimd.dma_start(qdup_r[0:H, :],
                                    gqk[2 * H:3 * H, bass.ds(qoff, RW)])
                nc.gpsimd.dma_start(qdup_r[H:P, :],
                                    gqk[2 * H:3 * H, bass.ds(qoff, RW)])
                v_rect = big.tile([P, NT, VROW], BF16, tag="vrect")
                nc.vector.memset(v_rect[:, :, H:H + 1], 1.0)
                nc.sync.dma_start(
                    v_rect[:, :, 0:H],
                    gv2[0:HALF, :].rearrange("(st p) h -> p st h", p=P))

                attT_rect = big.tile([P, NT * RW], BF16, tag="att_rect")
                for jp in range(8):
                    i0, i1 = 2 * jp, 2 * jp + 1
                    for g in range(2):
                        sl = slice(g * 512, (g + 1) * 512)
                        pa = ps.tile([P, 512], F32, tag="a")
                        pb = ps.tile([P, 512], F32, tag="b")
                        nc.tensor.matmul(pa[:], kdup_r[0:H, i0 * P:(i0 + 1) * P],
                                         qdup_r[0:H, sl], start=True, stop=True)
                        nc.tensor.matmul(pb[:], kdup_r[H:P, i1 * P:(i1 + 1) * P],
                                         qdup_r[H:P, sl], start=True, stop=True)
                        nc.scalar.activation(
                            attT_rect[:, i0 * RW + g * 512:i0 * RW + g * 512 + 512],
                            pa[:], EXP, scale=SCALE)
                        nc.scalar.activation(
                            attT_rect[:, i1 * RW + g * 512:i1 * RW + g * 512 + 512],
                            pb[:], EXP, scale=SCALE)

            if 7 in cur:
                # ---- rect AV (transposed, num|den via ones row) ----
                rectnd = big.tile([H + 1, RW], F32, tag="rectnd")
                for g in range(2):
                    pav = ps.tile([H + 1, 512], F32, tag="a")
                    for st in range(NT):
                        nc.tensor.matmul(
                            pav[:], v_rect[:, st, 0:H + 1],
                            attT_rect[:, st * RW + g * 512:st * RW + g * 512 + 512],
                            start=(st == 0), stop=(st == NT - 1))
                    nc.vector.tensor_copy(rectnd[:, g * 512:(g + 1) * 512], pav[:])

            if 8 in cur:
                # ---- partial exchange (flies during the triangle AV) ----
                ndb = dram.tile([H + 1, RW], F32)
                nc.gpsimd.dma_start(ndb[:], rectnd[:])
                gnd = dram.tile([2 * (H + 1), RW], F32)
                nc.gpsimd.collective_compute(
                    "AllGather", mybir.AluOpType.bypass, replica_groups=PAIRS,
                    ins=[ndb.opt()], outs=[gnd.opt()])

            if 6 in cur:
                # ---- triangle AV (transposed accumulation) ----
                trind = big.tile([H + 1, HALF], F32, tag="trind")
                for g in range(4):
                    pav = ps.tile([H + 1, 512], F32, tag="b")
                    last = 4 * g + 3
                    for st in range(last + 1):
                        cs = max(512 * g, P * st)
                        w = 512 * g + 512 - cs
                        col = TRI_OFF[st] + cs - TRI_BASE[st]
                        nc.tensor.matmul(pav[:, cs - 512 * g:512],
                                         v_own[:, st, 0:H + 1],
                                         attT_tri[:, col:col + w],
                                         start=(st == 0), stop=(st == last))
                    nc.vector.tensor_copy(trind[:, g * 512:(g + 1) * 512], pav[:])

            if 9 in cur:
                # ---- merge rect partials (upper core only) + divide ----
                nc.gpsimd.dma_start(trind[:, 0:RW], gnd[0:H + 1, :],
                                    accum_op=mybir.AluOpType.add, cond=is_h1)
                nc.gpsimd.dma_start(trind[:, RW:HALF], gnd[H + 1:2 * (H + 1), :],
                                    accum_op=mybir.AluOpType.add, cond=is_h1)
                nc.vector.reciprocal(trind[H:H + 1, :], trind[H:H + 1, :])
                recip_r = big.tile([H + 1, HALF], F32R, tag="recip_r")
                nc.vector.tensor_copy(recip_r[H:H + 1, :], trind[H:H + 1, :])
                out_sb = big.tile([H, HALF], F32, tag="out_sb")
                for g in range(4):
                    sl = slice(g * 512, (g + 1) * 512)
                    pbc = ps.tile([H, 512], F32, tag="v")
                    nc.tensor.matmul(pbc[:], ones_r[H:H + 1, :],
                                     recip_r[H:H + 1, sl],
                                     start=True, stop=True)
                    nc.vector.tensor_mul(out_sb[:, sl], trind[0:H, sl], pbc[:])
                nc.sync.dma_start(outT[:], out_sb[:])

            if DEBUG_DUMPS:
                def dump(name, src_fn, shape, dt):
                    try:
                        src_ap = src_fn()
                    except NameError:
                        return
                    o = nc.dram_tensor(name, shape, dt,
                                       kind="ExternalOutput").ap()
                    nc.sync.dma_start(o[:], src_ap)
                dump("d_qdup", lambda: qdup[:], [P, HALF], F32R)
                dump("d_kdup", lambda: kdup[:], [P, HALF], F32R)
                dump("d_vown", lambda: v_own[:].rearrange("p a b -> p (a b)"),
                     [P, NT * VROW], BF16)
                dump("d_atttri0", lambda: attT_tri[:, 0:2048], [P, 2048], BF16)
                dump("d_attrect0", lambda: attT_rect[:, 0:RW], [P, RW], BF16)
                dump("d_rectnd", lambda: rectnd[:], [H + 1, RW], F32)
                dump("d_trind", lambda: trind[:], [H + 1, HALF], F32)

    nc.compile()
    return nc


def make_in_maps(x, Wq, Wk, Wv):
    x = np.asarray(x, dtype=np.float32)
    Wq = np.asarray(Wq, dtype=np.float32)
    Wk = np.asarray(Wk, dtype=np.float32)
    Wv = np.asarray(Wv, dtype=np.float32)
    # S^T layout: partition=s, free=t; allowed s<=t -> tri[s,t]=0 iff s<=t
    tri = np.where(np.arange(P)[:, None] <= np.arange(P)[None, :], 0.0,
                   NEG).astype(np.float32)
    in_maps = []
    for c in range(N_CORES):
        b, h = c // 2, c % 2
        in_maps.append({
            "x": np.ascontiguousarray(x[b, h * HALF:(h + 1) * HALF, :]),
            "wq": Wq, "wk": Wk, "wv": Wv,
            "trimask": tri,
        })
    return in_maps


def kernel(x, Wq, Wk, Wv):
    if "nc" not in _CACHE:
        _CACHE["nc"] = build()
    nc = _CACHE["nc"]
    in_maps = make_in_maps(x, Wq, Wk, Wv)
    res = None
    for attempt in range(4):
        try:
            res = run_bass_kernel_spmd(nc, in_maps, list(range(N_CORES)))
            break
        except Exception:
            if attempt == 3:
                raise
            import time as _time
            _time.sleep(5)
    out = np.empty((B, T, H), np.float32)
    for c in range(N_CORES):
        b, h = c // 2, c % 2
        out[b, h * HALF:(h + 1) * HALF, :] = res.results[c]["outT"].T
    return out


# revision 24
# speedup vs baseline: 2.3618x; 1.1203x over previous
"""Causal single-head attention (B=4, T=4096, C=1024, H=64) on 8 TRN2 NeuronCores.

Sharding: core = 2*b + h handles batch b, t-half h (rows [h*2048, (h+1)*2048)).
Uniform SPMD program per core:
  - triangle: causal attention within the own t-half (s, t both in own half)
  - rect: S^T[s in [0,2048), t in [2048+off, 2048+off+1024)], off = (pid%2)*1024
    (lower-half keys attending into upper-half queries, t-split across the pair)

v2 design:
  - x loaded per t-tile with SWDGE cast f32->bf16 into SBUF, then one 3-D xbar
    SBUF->SBUF DMA transpose per tile: xT[p, cb, t] = x[t, 128*cb + p].
  - v projected in vT form (wide moving operand), then xbar-transposed into
    v_own[s, h] layout with an appended ones column (softmax denominator).
  - QK^T runs as row-packed pairs: two concurrent K=64 matmuls on row groups
    (0,0)/(64,0), needing q/k duplicated into both partition halves.
  - AV computed transposed: outT[h, t] += v'[s, h].T @ attT[s, t] with 512-wide
    bf16 moving operand; row 64 of the accumulator is the denominator.
  - Rect partials pair-AllGathered; merged into trind by a conditional
    accumulate-DMA (only on the upper-half core). Final divide via reciprocal
    of the den row + K=1 outer-product broadcast matmul. Output written as
    outT [64, 2048]; the host transposes.
Softmax uses no max-subtraction (logits are O(6)).
"""
import sys

sys.path.insert(0, "/opt/trn_rl_repo")

from contextlib import ExitStack

import numpy as np

import concourse.bass as bass
import concourse.mybir as mybir
import concourse.tile as tile
from concourse import bacc
from concourse.bass_utils import run_bass_kernel_spmd

B, T, C, H = 4, 4096, 1024, 64
P = 128
HALF = T // 2              # 2048 rows per core
NB_C = C // P              # 8 contraction tiles
NT = HALF // P             # 16 own t/s tiles
RW = 1024                  # rect t-width per core
SCALE = float(H) ** -0.5
NEG = -1e9
F32, F32R, BF16 = mybir.dt.float32, mybir.dt.float32r, mybir.dt.bfloat16
N_CORES = 8
PAIRS = [[2 * b, 2 * b + 1] for b in range(B)]

# triangle attT storage: s-tile i holds t-cols [256*(i//2), 2048)
TRI_BASE = [256 * (i // 2) for i in range(NT)]
TRI_W = [HALF - b for b in TRI_BASE]
TRI_OFF = np.concatenate([[0], np.cumsum(TRI_W)]).tolist()
TRI_TOTAL = TRI_OFF[-1]  # 18432
VROW = H + 16               # v_own/v_rect row stride: 160B, 32B-aligned

_CACHE = {}
BODY_REPEAT = 1            # for differential timing in test.py
PHASES = set(range(1, 10))  # ablation for phase timing
SCHEDULE = None            # list of phase-sets, one body emission each
DEBUG_DUMPS = False        # emit intermediate tensors as extra outputs


def build():
    nc = bacc.Bacc("TRN2", target_bir_lowering=False, debug=False,
                   num_devices=N_CORES)
    x = nc.dram_tensor("x", [HALF, C], F32, kind="ExternalInput").ap()
    wq = nc.dram_tensor("wq", [C, H], F32, kind="ExternalInput").ap()
    wk = nc.dram_tensor("wk", [C, H], F32, kind="ExternalInput").ap()
    wv = nc.dram_tensor("wv", [C, H], F32, kind="ExternalInput").ap()
    trimask = nc.dram_tensor("trimask", [P, P], F32, kind="ExternalInput").ap()
    outT = nc.dram_tensor("outT", [H, HALF], F32, kind="ExternalOutput").ap()

    EXP = mybir.ActivationFunctionType.Exp

    with tile.TileContext(nc) as tc, ExitStack() as ctx:
        big = ctx.enter_context(tc.tile_pool(name="big", bufs=1))
        stage = ctx.enter_context(tc.tile_pool(name="stage", bufs=2))
        ps = ctx.enter_context(tc.tile_pool(name="ps", bufs=2, space="PSUM"))
        dram = ctx.enter_context(tc.tile_pool(name="dram", bufs=1, space="DRAM"))

        # ---- constants ----
        tri_sb = big.tile([P, P], F32, tag="tri")
        nc.sync.dma_start(tri_sb[:], trimask[:])
        wqk_sb = big.tile([P, NB_C, 2 * H], BF16, tag="wqk")
        nc.gpsimd.dma_start(wqk_sb[:, :, 0:H], wq.rearrange("(cb p) h -> p cb h", p=P))
        nc.gpsimd.dma_start(wqk_sb[:, :, H:2 * H], wk.rearrange("(cb p) h -> p cb h", p=P))
        wv_sb = big.tile([P, NB_C, H], BF16, tag="wv")
        nc.gpsimd.dma_start(wv_sb[:], wv.rearrange("(cb p) h -> p cb h", p=P))
        # ones row on partition H (=64): lhsT for the den-broadcast matmul must
        # share the contraction partition with the den row of trind
        ones_sb = big.tile([H + 1, H], F32, tag="ones")
        nc.vector.memset(ones_sb[H:H + 1, :], 1.0)
        ones_r = big.tile([H + 1, H], F32R, tag="ones_r")
        nc.vector.tensor_copy(ones_r[H:H + 1, :], ones_sb[H:H + 1, :])

        pid = nc.partition_id(engines=[mybir.EngineType.Pool])
        qoff = (pid % 2) * RW
        is_h1 = pid % 2

        schedule = SCHEDULE if SCHEDULE is not None else [PHASES] * BODY_REPEAT
        for _rep in range(len(schedule)):
            cur = schedule[_rep]
            if 1 in cur:
                # ---- x load (cast bf16, 512-row quarters) + xbar per t-tile
                # (transposes alternate across both HWDGE rings) ----
                xT = big.tile([P, NB_C, HALF], BF16, tag="xT")
                for q in range(4):
                    xs = stage.tile([P, 4, C], BF16, tag="xs")
                    nc.gpsimd.dma_start(
                        xs[:], x[q * 512:(q + 1) * 512, :].rearrange(
                            "(a p) c -> p a c", p=P))
                    for a in range(4):
                        tt = 4 * q + a
                        nc.sync.dma_start(xT[:, :, tt * P:(tt + 1) * P],
                                          xs[:, a, :], transpose=True)

            if 2 in cur:
                # ---- projections: q|k packed wide; v direct in [t, h] ----
                qdup = big.tile([P, HALF], F32R, tag="qdup")
                kdup = big.tile([P, HALF], F32R, tag="kdup")
                for tg in range(4):
                    sl = slice(tg * 512, (tg + 1) * 512)
                    pqk = ps.tile([P, 512], F32, tag="a")
                    for cb in range(NB_C):
                        nc.tensor.matmul(pqk[:], wqk_sb[:, cb, :],
                                         xT[:, cb, sl],
                                         start=(cb == 0), stop=(cb == NB_C - 1))
                    nc.vector.tensor_copy(qdup[0:H, sl], pqk[0:H, :])
                    nc.vector.tensor_copy(kdup[H:P, sl], pqk[H:P, :])
                v_own = big.tile([P, NT, VROW], BF16, tag="vown")
                nc.vector.memset(v_own[:, :, H:H + 1], 1.0)
                for st in range(NT):
                    pv = ps.tile([P, H], F32, tag="v")
                    for cb in range(NB_C):
                        nc.tensor.matmul(pv[:],
                                         xT[:, cb, st * P:(st + 1) * P],
                                         wv_sb[:, cb, :],
                                         start=(cb == 0), stop=(cb == NB_C - 1))
                    nc.vector.tensor_copy(v_own[:, st, 0:H], pv[:])
                # q|k and v out to DRAM (feeds the AllGather and the dups)
                qkb = dram.tile([2 * H, HALF], F32R)
                vb2 = dram.tile([HALF, H], BF16)
                nc.gpsimd.dma_start(qkb[0:H, :], qdup[0:H, :])
                nc.gpsimd.dma_start(qkb[H:2 * H, :], kdup[H:P, :])
                nc.gpsimd.dma_start(vb2.rearrange("(st p) h -> p st h", p=P),
                                    v_own[:, :, 0:H])
                # cross-partition dups via DRAM
                nc.sync.dma_start(qdup[H:P, :], qkb[0:H, :])
                nc.sync.dma_start(kdup[0:H, :], qkb[H:2 * H, :])

            if 3 in cur:
                # ---- pair collectives: gather q|k and v ----
                gqk = dram.tile([4 * H, HALF], F32R)
                gv2 = dram.tile([T, H], BF16)
                for src, dst in ((qkb, gqk), (vb2, gv2)):
                    nc.gpsimd.collective_compute(
                        "AllGather", mybir.AluOpType.bypass,
                        replica_groups=PAIRS,
                        ins=[src.opt()], outs=[dst.opt()])

            if 4 in cur:
                # ---- triangle QK^T as row-packed pairs + exp ----
                attT_tri = big.tile([P, TRI_TOTAL], BF16, tag="att_tri")
                for j in range(8):
                    base = 256 * j
                    i0, i1 = 2 * j, 2 * j + 1
                    for c0 in range(base, HALF, 512):
                        w = min(512, HALF - c0)
                        pa = ps.tile([P, 512], F32, tag="a")
                        pb = ps.tile([P, 512], F32, tag="b")
                        nc.tensor.matmul(pa[:, 0:w],
                                         kdup[0:H, i0 * P:(i0 + 1) * P],
                                         qdup[0:H, c0:c0 + w],
                                         start=True, stop=True)
                        nc.tensor.matmul(pb[:, 0:w],
                                         kdup[H:P, i1 * P:(i1 + 1) * P],
                                         qdup[H:P, c0:c0 + w],
                                         start=True, stop=True)
                        if c0 == base:
                            nc.vector.tensor_add(pa[:, 0:P], pa[:, 0:P], tri_sb[:])
                            nc.vector.tensor_add(pb[:, P:2 * P], pb[:, P:2 * P],
                                                 tri_sb[:])
                        d0 = c0 - base
                        nc.scalar.activation(
                            attT_tri[:, TRI_OFF[i0] + d0:TRI_OFF[i0] + d0 + w],
                            pa[:, 0:w], EXP, scale=SCALE)
                        nc.scalar.activation(
                            attT_tri[:, TRI_OFF[i1] + d0:TRI_OFF[i1] + d0 + w],
                            pb[:, 0:w], EXP, scale=SCALE)

            if 5 in cur:
                # ---- rect operands (from gathered) + rect QK^T pairs ----
                kdup_r = big.tile([P, HALF], F32R, tag="kdup_r")
                nc.sync.dma_start(kdup_r[0:H, :], gqk[H:2 * H, :])
                nc.sync.dma_start(kdup_r[H:P, :], gqk[H:2 * H, :])
                qdup_r = big.tile([P, RW], F32R, tag="qdup_r")
                nc.gpsimd.dma_start(qdup_r[0:H, :],
                                    gqk[2 * H:3 * H, bass.ds(qoff, RW)])
                nc.gpsimd.dma_start(qdup_r[H:P, :],
                                    gqk[2 * H:3 * H, bass.ds(qoff, RW)])
                v_rect = big.tile([P, NT, VROW], BF16, tag="vrect")
                nc.vector.memset(v_rect[:, :, H:H + 1], 1.0)
                nc.sync.dma_start(
                    v_rect[:, :, 0:H],
                    gv2[0:HALF, :].rearrange("(st p) h -> p st h", p=P))

                attT_rect = big.tile([P, NT * RW], BF16, tag="att_rect")
                for jp in range(8):
                    i0, i1 = 2 * jp, 2 * jp + 1
                    for g in range(2):
                        sl = slice(g * 512, (g + 1) * 512)
                        pa = ps.tile([P, 512], F32, tag="a")
                        pb = ps.tile([P, 512], F32, tag="b")
                        nc.tensor.matmul(pa[:], kdup_r[0:H, i0 * P:(i0 + 1) * P],
                                         qdup_r[0:H, sl], start=True, stop=True)
                        nc.tensor.matmul(pb[:], kdup_r[H:P, i1 * P:(i1 + 1) * P],
                                         qdup_r[H:P, sl], start=True, stop=True)
                        nc.scalar.activation(
                            attT_rect[:, i0 * RW + g * 512:i0 * RW + g * 512 + 512],
                            pa[:], EXP, scale=SCALE)
                        nc.scalar.activation(
                            attT_rect[:, i1 * RW + g * 512:i1 * RW + g * 512 + 512],
                            pb[:], EXP, scale=SCALE)

            if 7 in cur:
                # ---- rect AV (transposed, num|den via ones row) ----
                rectnd = big.tile([H + 1, RW], F32, tag="rectnd")
                for g in range(2):
                    pav = ps.tile([H + 1, 512], F32, tag="a")
                    for st in range(NT):
                        nc.tensor.matmul(
                            pav[:], v_rect[:, st, 0:H + 1],
                            attT_rect[:, st * RW + g * 512:st * RW + g * 512 + 512],
                            start=(st == 0), stop=(st == NT - 1))
                    nc.vector.tensor_copy(rectnd[:, g * 512:(g + 1) * 512], pav[:])

            if 8 in cur:
                # ---- partial exchange (flies during the triangle AV) ----
                ndb = dram.tile([H + 1, RW], F32)
                nc.gpsimd.dma_start(ndb[:], rectnd[:])
                gnd = dram.tile([2 * (H + 1), RW], F32)
                nc.gpsimd.collective_compute(
                    "AllGather", mybir.AluOpType.bypass, replica_groups=PAIRS,
                    ins=[ndb.opt()], outs=[gnd.opt()])

            if 6 in cur:
                # ---- triangle AV (transposed accumulation) ----
                trind = big.tile([H + 1, HALF], F32, tag="trind")
                for g in range(4):
                    pav = ps.tile([H + 1, 512], F32, tag="b")
                    last = 4 * g + 3
                    for st in range(last + 1):
                        cs = max(512 * g, P * st)
                        w = 512 * g + 512 - cs
                        col = TRI_OFF[st] + cs - TRI_BASE[st]
                        nc.tensor.matmul(pav[:, cs - 512 * g:512],
                                         v_own[:, st, 0:H + 1],
                                         attT_tri[:, col:col + w],
                                         start=(st == 0), stop=(st == last))
                    nc.vector.tensor_copy(trind[:, g * 512:(g + 1) * 512], pav[:])

            if 9 in cur:
                # ---- merge rect partials (upper core only) + divide ----
                nc.gpsimd.dma_start(trind[:, 0:RW], gnd[0:H + 1, :],
                                    accum_op=mybir.AluOpType.add, cond=is_h1)
                nc.gpsimd.dma_start(trind[:, RW:HALF], gnd[H + 1:2 * (H + 1), :],
                                    accum_op=mybir.AluOpType.add, cond=is_h1)
                nc.vector.reciprocal(trind[H:H + 1, :], trind[H:H + 1, :])
                recip_r = big.tile([H + 1, HALF], F32R, tag="recip_r")
                nc.vector.tensor_copy(recip_r[H:H + 1, :], trind[H:H + 1, :])
                out_sb = big.tile([H, HALF], F32, tag="out_sb")
                for g in range(4):
                    sl = slice(g * 512, (g + 1) * 512)
                    pbc = ps.tile([H, 512], F32, tag="v")
                    nc.tensor.matmul(pbc[:], ones_r[H:H + 1, :],
                                     recip_r[H:H + 1, sl],
                                     start=True, stop=True)
                    nc.vector.tensor_mul(out_sb[:, sl], trind[0:H, sl], pbc[:])
                nc.sync.dma_start(outT[:], out_sb[:])

            if DEBUG_DUMPS:
                def dump(name, src_fn, shape, dt):
                    try:
                        src_ap = src_fn()
                    except NameError:
                        return
                    o = nc.dram_tensor(name, shape, dt,
                                       kind="ExternalOutput").ap()
                    nc.sync.dma_start(o[:], src_ap)
                dump("d_qdup", lambda: qdup[:], [P, HALF], F32R)
                dump("d_kdup", lambda: kdup[:], [P, HALF], F32R)
                dump("d_vown", lambda: v_own[:].rearrange("p a b -> p (a b)"),
                     [P, NT * VROW], BF16)
                dump("d_atttri0", lambda: attT_tri[:, 0:2048], [P, 2048], BF16)
                dump("d_attrect0", lambda: attT_rect[:, 0:RW], [P, RW], BF16)
                dump("d_rectnd", lambda: rectnd[:], [H + 1, RW], F32)
                dump("d_trind", lambda: trind[:], [H + 1, HALF], F32)

    nc.compile()
    return nc


def make_in_maps(x, Wq, Wk, Wv):
    x = np.asarray(x, dtype=np.float32)
    Wq = np.asarray(Wq, dtype=np.float32)
    Wk = np.asarray(Wk, dtype=np.float32)
    Wv = np.asarray(Wv, dtype=np.float32)
    # S^T layout: partition=s, free=t; allowed s<=t -> tri[s,t]=0 iff s<=t
    tri = np.where(np.arange(P)[:, None] <= np.arange(P)[None, :], 0.0,
                   NEG).astype(np.float32)
    in_maps = []
    for c in range(N_CORES):
        b, h = c // 2, c % 2
        in_maps.append({
            "x": np.ascontiguousarray(x[b, h * HALF:(h + 1) * HALF, :]),
            "wq": Wq, "wk": Wk, "wv": Wv,
            "trimask": tri,
        })
    return in_maps


def kernel(x, Wq, Wk, Wv):
    if "nc" not in _CACHE:
        _CACHE["nc"] = build()
    nc = _CACHE["nc"]
    in_maps = make_in_maps(x, Wq, Wk, Wv)
    res = None
    for attempt in range(4):
        try:
            res = run_bass_kernel_spmd(nc, in_maps, list(range(N_CORES)))
            break
        except Exception:
            if attempt == 3:
                raise
            import time as _time
            _time.sleep(5)
    out = np.empty((B, T, H), np.float32)
    for c in range(N_CORES):
        b, h = c // 2, c % 2
        out[b, h * HALF:(h + 1) * HALF, :] = res.results[c]["outT"].T
    return out
